# revision 21
# baseline (speedup 1.0000x reference)
"""Multi-head attention (B=4, S=2048, D=1024, H=16, dk=dv=64) on 8 TRN2 cores.

Sharding: core c = 2*b + hg handles batch b = c//2 and heads
[hg*8, hg*8+8). Each core computes a partial output
(its 8 heads' contribution through Wo); the host adds the two partials
per batch.

Per-core pipeline (matmul inputs bf16, PSUM fp32, ScalarE exp paces the
attention phase at ~1.5us per [128,1024] tile):
  Prefix: stage kT/vT with 8 big DMAs each; k-projection with
    c-outer/qb-inner loops (stationary weight reuse, 4 PSUM banks);
    q-projection for qb0. khT[p] stores the head PAIR packed
    (rows 0-63 = h0 dk, 64-127 = h1 dk) - same as qhT.
  Attention per (qb, pair): per key chunk kc, two K=64 scores matmuls
    (tile_position auto (0,0)/(64,0) - concurrent row tiles on HW) into
    one [128,1024] PSUM tile (h0 cols 0:512, h1 512:1024); one exp ACT
    per tile; two mix matmuls (lhsT = vh_aug [128 keys, 65] with a
    mask/ones sums column) accumulating into mixP/mixR.
  Normalize per pair: DVE casts PSUM sums rows (row 64) to fp16 at
    partition 64, one DMA moves both heads' sums to partition 0, two
    K=1 fp16 broadcast matmuls replicate them across 64 partitions,
    DVE reciprocal + multiply produce normalized bf16 mixT; h1 is
    DMA-shifted to partitions 64-127.
  v-projection is interleaved into pair 0's score loop; q-projection
    (qb+1) and Wo (qb-1) groups fill interleave slots in later pairs so
    the PE works inside the ScalarE exp slack.
  Wo: out += normT.T @ Wo accumulated over the 4 pairs; DVE evac; DMA.
"""

import numpy as np

B, S, D = 4, 2048, 1024
H, DK, DV = 16, 64, 64
HC = 8          # heads per core
NP = HC // 2    # head pairs per core
NCORES = 8
NC_CHUNKS = D // 128    # 8 contraction chunks over D
NKC = S // 128          # 16 key chunks
NQB = S // 512          # 4 query blocks
VW = HC * 65            # vh storage: 65 cols per head (dv | mask)

_COMPILED = {}


def _build_nc():
    import concourse.tile as tile
    from concourse import bacc, mybir
    from contextlib import ExitStack

    F32 = mybir.dt.float32
    F16 = mybir.dt.float16
    BF16 = mybir.dt.bfloat16
    EXP = mybir.ActivationFunctionType.Exp

    nc = bacc.Bacc("TRN2", target_bir_lowering=False, debug=False,
                   num_devices=NCORES)

    qT = nc.dram_tensor("qT", [D, S], BF16, kind="ExternalInput").ap()
    kT = nc.dram_tensor("kT", [D, S], BF16, kind="ExternalInput").ap()
    vT = nc.dram_tensor("vT", [D, S], BF16, kind="ExternalInput").ap()
    wq = nc.dram_tensor("wq", [D, HC * DK], BF16, kind="ExternalInput").ap()
    wk = nc.dram_tensor("wk", [D, HC * DK], BF16, kind="ExternalInput").ap()
    wv = nc.dram_tensor("wv", [D, HC * DV], BF16, kind="ExternalInput").ap()
    wo = nc.dram_tensor("wo", [HC * DV, D], BF16, kind="ExternalInput").ap()
    maskr = nc.dram_tensor("maskr", [128, NKC], F32, kind="ExternalInput").ap()
    out = nc.dram_tensor("out", [S, D], F32, kind="ExternalOutput").ap()

    with tile.TileContext(nc) as tc:
        with ExitStack() as ctx:
            const_pool = ctx.enter_context(tc.tile_pool(name="const", bufs=1))
            w_pool = ctx.enter_context(tc.tile_pool(name="weights", bufs=1))
            act_pool = ctx.enter_context(tc.tile_pool(name="acts", bufs=1))

            mask_sb = const_pool.tile([128, NKC], F32)
            ones_sb = const_pool.tile([128, HC], BF16)
            sel16 = const_pool.tile([1, DV], F16)

            # weights staged chunk-major: col block c holds rows c*128..+128
            wq_sb = w_pool.tile([128, NC_CHUNKS * 512], BF16, tag="wq")
            wv_sb = w_pool.tile([128, NC_CHUNKS * 512], BF16, tag="wv")
            wo_sb = w_pool.tile([128, NP * 1024], BF16, tag="wo")

            # persistent activations (pair-packed: rows 0-63 h0, 64-127 h1)
            qhT = [act_pool.tile([128, S], BF16, tag=f"qhT{p}", name=f"qhT{p}")
                   for p in range(NP)]
            khT = [act_pool.tile([128, S], BF16, tag=f"khT{p}", name=f"khT{p}")
                   for p in range(NP)]
            vhs = [act_pool.tile([128, VW], BF16, tag=f"vh{t}", name=f"vh{t}")
                   for t in range(NKC)]

            # attention pools created BEFORE the staging scopes so that the
            # staging scopes can close in LIFO order mid-kernel.
            qst_pool = ctx.enter_context(
                tc.tile_pool(name="qstage", bufs=12))
            sc_pool = ctx.enter_context(
                tc.tile_pool(name="scpsum", bufs=3, space="PSUM"))
            mx_pool = ctx.enter_context(
                tc.tile_pool(name="mxpsum", bufs=2, space="PSUM"))

            def sh_tile():
                # aux PSUM comes from the sc pool (half of a 2-bank tile);
                # held briefly so the scores rotation keeps 2+ buffers free
                return sc_pool.tile([128, 1024], F32, tag="sc",
                                    name="aux")[:, 0:512]
            exp_pool = ctx.enter_context(tc.tile_pool(name="exp", bufs=6))
            norm_pool = ctx.enter_context(tc.tile_pool(name="norm", bufs=9))
            tmp_pool = ctx.enter_context(tc.tile_pool(name="tmp", bufs=3))
            s64_pool = ctx.enter_context(tc.tile_pool(name="s64", bufs=2))
            srow_pool = ctx.enter_context(tc.tile_pool(name="srow", bufs=2))
            out_pool = ctx.enter_context(tc.tile_pool(name="outsb", bufs=2))

            # ---- issue all input DMAs up front (big tiles first) ----
            # vstage allocated first, kstage second: kstage scope closes
            # right after the prefix (LIFO), vstage after pair 0's v-proj.
            vst_ctx = ExitStack()
            vst_pool = vst_ctx.enter_context(
                tc.tile_pool(name="vstage", bufs=1))
            vstg = [vst_pool.tile([128, S], BF16, tag=f"vst{c}",
                                  name=f"vst{c}") for c in range(NC_CHUNKS)]
            kst_ctx = ExitStack()
            kst_pool = kst_ctx.enter_context(
                tc.tile_pool(name="kstage", bufs=1))
            wk_sb = kst_pool.tile([128, NC_CHUNKS * 512], BF16, tag="wk")
            kstg = [kst_pool.tile([128, S], BF16, tag=f"kst{c}",
                                  name=f"kst{c}") for c in range(NC_CHUNKS)]
            for c in range(NC_CHUNKS):
                nc.sync.dma_start(wk_sb[:, c * 512:(c + 1) * 512],
                                  wk[c * 128:(c + 1) * 128, :])
                nc.sync.dma_start(kstg[c][:], kT[c * 128:(c + 1) * 128, :])

            def stage_q(qb):
                stg = []
                for c in range(NC_CHUNKS):
                    t = qst_pool.tile([128, 512], BF16, tag="qst",
                                      name=f"qst{qb}_{c}")
                    nc.sync.dma_start(
                        t[:], qT[c * 128:(c + 1) * 128,
                                 qb * 512:(qb + 1) * 512])
                    stg.append(t)
                return stg

            for c in range(NC_CHUNKS):
                nc.sync.dma_start(wq_sb[:, c * 512:(c + 1) * 512],
                                  wq[c * 128:(c + 1) * 128, :])
            qstg0 = stage_q(0)
            nc.sync.dma_start(mask_sb[:], maskr[:])
            for c in range(NC_CHUNKS):
                nc.sync.dma_start(wv_sb[:, c * 512:(c + 1) * 512],
                                  wv[c * 128:(c + 1) * 128, :])
            for p in range(NP):
                nc.sync.dma_start(wo_sb[:, p * 1024:(p + 1) * 1024],
                                  wo[p * 128:(p + 1) * 128, :])
            for c in range(NC_CHUNKS):
                nc.sync.dma_start(vstg[c][:], vT[c * 128:(c + 1) * 128, :])
            nc.vector.memset(ones_sb[:], 1.0)
            nc.vector.memset(sel16[:], 1.0)

            # ---- prefix: k projection (c-outer, stationary reuse) ----
            # kps tiles come from the attention sc pool ([128,1024] holds
            # two query blocks side by side).
            for p in range(NP):
                kpsA = sc_pool.tile([128, 1024], F32, tag="sc",
                                    name=f"kpsA{p}")
                kpsB = sc_pool.tile([128, 1024], F32, tag="sc",
                                    name=f"kpsB{p}")
                halves = [kpsA[:, 0:512], kpsA[:, 512:1024],
                          kpsB[:, 0:512], kpsB[:, 512:1024]]
                for c in range(NC_CHUNKS):
                    wsl = wk_sb[:, c * 512 + p * 128:
                                c * 512 + (p + 1) * 128]
                    for qb in range(NQB):
                        nc.tensor.matmul(
                            halves[qb],
                            lhsT=wsl,
                            rhs=kstg[c][:, qb * 512:(qb + 1) * 512],
                            start=(c == 0), stop=(c == NC_CHUNKS - 1))
                nc.vector.tensor_copy(khT[p][:, 0:1024], kpsA[:])
                nc.vector.tensor_copy(khT[p][:, 1024:2048], kpsB[:])
            # q projection for qb0
            for p in range(NP):
                qps = sh_tile()
                for c in range(NC_CHUNKS):
                    nc.tensor.matmul(
                        qps[:],
                        lhsT=wq_sb[:, c * 512 + p * 128:
                                   c * 512 + (p + 1) * 128],
                        rhs=qstg0[c][:],
                        start=(c == 0), stop=(c == NC_CHUNKS - 1))
                nc.vector.tensor_copy(qhT[p][:, 0:512], qps[:])
            kst_ctx.close()   # free kT staging (32KB/partition)

            def vproj_chunk(t):
                """Project v for token chunk t into vhs[t] (all 8 heads)."""
                vps = sh_tile()
                for c in range(NC_CHUNKS):
                    nc.tensor.matmul(
                        vps[:],
                        lhsT=vstg[c][:, t * 128:(t + 1) * 128],
                        rhs=wv_sb[:, c * 512:(c + 1) * 512],
                        start=(c == 0), stop=(c == NC_CHUNKS - 1))
                dst_dv = vhs[t][:, 0:VW].rearrange(
                    "p (h x) -> p h x", x=65)[:, :, 0:DV]
                src_dv = vps[:].rearrange("p (h x) -> p h x", x=DV)
                nc.vector.tensor_scalar_mul(dst_dv, src_dv,
                                            mask_sb[:, t:t + 1])
                dst_m = vhs[t][:, 0:VW].rearrange(
                    "p (h x) -> p h x", x=65)[:, :, DV:DV + 1]
                src_m = ones_sb[:, 0:HC].rearrange("p (h x) -> p h x", x=1)
                nc.vector.tensor_scalar_mul(dst_m, src_m,
                                            mask_sb[:, t:t + 1])

            def qproj_group(qb, p, stg):
                """Project q for (qb, pair p) into qhT[p]."""
                qps = sh_tile()
                for c in range(NC_CHUNKS):
                    nc.tensor.matmul(
                        qps[:],
                        lhsT=wq_sb[:, c * 512 + p * 128:
                                   c * 512 + (p + 1) * 128],
                        rhs=stg[c][:],
                        start=(c == 0), stop=(c == NC_CHUNKS - 1))
                nc.vector.tensor_copy(qhT[p][:, qb * 512:(qb + 1) * 512],
                                      qps[:])

            def wo_group(qb, tt, dh, normT):
                """One Wo output tile [128 q, 512 d] accumulated over pairs."""
                wps = sh_tile()
                for p in range(NP):
                    nc.tensor.matmul(
                        wps[:],
                        lhsT=normT[p][:, tt * 128:(tt + 1) * 128],
                        rhs=wo_sb[:, p * 1024 + dh * 512:
                                  p * 1024 + (dh + 1) * 512],
                        start=(p == 0), stop=(p == NP - 1))
                osb = out_pool.tile([128, 512], F32, tag="osb")
                if qb == NQB - 1:
                    nc.scalar.copy(osb[:], wps[:])
                else:
                    nc.vector.tensor_copy(osb[:], wps[:])
                nc.sync.dma_start(
                    out[qb * 512 + tt * 128:qb * 512 + (tt + 1) * 128,
                        dh * 512:(dh + 1) * 512], osb[:])

            # interleave slot queue: list of thunks
            pending = []
            post_norm = []

            def run_slot():
                if pending:
                    pending.pop(0)()

            for qb in range(NQB):
                normT = []
                for p in range(NP):
                    first = (qb == 0 and p == 0)
                    lag = 4 if first else 2
                    if p == NP - 1 and qb + 1 < NQB:
                        # queue q-projection for qb+1 into this pair's slots
                        stg = stage_q(qb + 1)
                        for pp in range(NP):
                            pending.append(
                                lambda qb=qb, pp=pp, stg=stg: qproj_group(
                                    qb + 1, pp, stg))
                    h0, h1 = 2 * p, 2 * p + 1
                    qful = qhT[p][:, qb * 512:(qb + 1) * 512]
                    mixP = mx_pool.tile([128, 512], F32, tag="mx")
                    mixR = mx_pool.tile([128, 512], F32, tag="mx")
                    exps = []

                    def mix_step(lk):
                        for h, mx in ((h0, mixP), (h1, mixR)):
                            nc.tensor.matmul(
                                mx[0:65, :],
                                lhsT=vhs[lk][:, h * 65:h * 65 + 65],
                                rhs=exps[lk][:, (h % 2) * 512:
                                             (h % 2) * 512 + 512],
                                start=(lk == 0), stop=(lk == NKC - 1))

                    # scores + exp per key chunk (two K=64 row tiles)
                    for kc in range(NKC):
                        ksl = slice(kc * 128, (kc + 1) * 128)
                        sc = sc_pool.tile([128, 1024], F32, tag="sc")
                        nc.tensor.matmul(
                            sc[:, 0:512],
                            lhsT=khT[p][0:64, ksl], rhs=qful[0:64, :],
                            start=True, stop=True)
                        nc.tensor.matmul(
                            sc[:, 512:1024],
                            lhsT=khT[p][64:128, ksl], rhs=qful[64:128, :],
                            start=True, stop=True)
                        ex = exp_pool.tile([128, 1024], BF16, tag="exp")
                        nc.scalar.activation(ex[:], sc[:], EXP)
                        exps.append(ex)
                        if first:
                            # v projection rides pair 0's score loop
                            vproj_chunk(kc)
                        if kc == 5 and post_norm:
                            post_norm.pop(0)()
                        if kc >= lag:
                            mix_step(kc - lag)
                        if (not first and kc % 4 == 3
                                and not (p == 0 and kc == 3)):
                            run_slot()
                    for lk in range(NKC - lag, NKC):
                        mix_step(lk)
                    if first:
                        vst_ctx.close()   # free vT staging
                    # ---- normalize (pre): sums rows -> fp16 -> partition 0
                    s64 = s64_pool.tile([128, 1024], F16, tag="s64")
                    nc.vector.tensor_copy(s64[64:65, 0:512], mixP[64:65, :])
                    nc.vector.tensor_copy(s64[64:65, 512:1024],
                                          mixR[64:65, :])
                    srow = srow_pool.tile([1, 1024], F16, tag="srow")
                    nc.gpsimd.dma_start(srow[0:1, :], s64[64:65, :])

                    def norm_post(mixP=mixP, mixR=mixR, srow=srow,
                                  normT=normT):
                        # broadcast (K=1 fp16 matmuls), recip, multiply;
                        # deferred into the next pair so the PE never waits
                        # on the sums DMA.
                        bct = sc_pool.tile([128, 1024], F32, tag="sc",
                                           name="bct")
                        bc0 = bct[:, 0:512]
                        bc1 = bct[:, 512:1024]
                        nc.tensor.matmul(bc0[0:DV, :], lhsT=sel16[:],
                                         rhs=srow[0:1, 0:512],
                                         start=True, stop=True)
                        nc.tensor.matmul(bc1[0:DV, :], lhsT=sel16[:],
                                         rhs=srow[0:1, 512:1024],
                                         start=True, stop=True)
                        rec0 = tmp_pool.tile([64, 512], F32, tag="rec")
                        rec1 = tmp_pool.tile([64, 512], F32, tag="rec")
                        nc.vector.reciprocal_approx_fast(rec0[:],
                                                         bc0[0:64, :])
                        nc.vector.reciprocal_approx_fast(rec1[:],
                                                         bc1[0:64, :])
                        nt = norm_pool.tile([128, 512], BF16, tag="norm")
                        normT.append(nt)
                        nc.vector.tensor_mul(nt[0:64, :], mixP[0:64, :],
                                             rec0[:])
                        sh1 = tmp_pool.tile([64, 512], BF16, tag="sh1")
                        nc.vector.tensor_mul(sh1[:], mixR[0:64, :], rec1[:])
                        nc.gpsimd.dma_start(nt[64:128, :], sh1[:])

                    if qb == NQB - 1 and p == NP - 1:
                        norm_post()
                    else:
                        post_norm.append(norm_post)

                # queue Wo for this qb into the next qb's interleave slots
                # (normT is filled lazily by deferred norm_post thunks; pass
                # the live list, complete by the time any wo_group runs)
                nt_list = normT
                for tt in range(4):
                    for dh in range(2):
                        pending.append(
                            lambda qb=qb, tt=tt, dh=dh, nt=nt_list: wo_group(
                                qb, tt, dh, nt))
                # last qb: drain all pending now
                if qb == NQB - 1:
                    while pending:
                        run_slot()

    nc.compile()
    return nc


def _get_nc():
    if "nc" not in _COMPILED:
        _COMPILED["nc"] = _build_nc()
    return _COMPILED["nc"]


def _shard_inputs(q, k, v, mask, Wq, Wk, Wv, Wo):
    """Build the per-core input maps (host-side layout prep)."""
    import ml_dtypes

    bf16 = ml_dtypes.bfloat16
    in_maps = []
    maskf = np.asarray(mask).astype(np.float32)
    q = np.asarray(q, np.float32)
    k = np.asarray(k, np.float32)
    v = np.asarray(v, np.float32)
    Wq = np.asarray(Wq, np.float32)
    Wk = np.asarray(Wk, np.float32)
    Wv = np.asarray(Wv, np.float32)
    Wo = np.asarray(Wo, np.float32)
    scale = np.float32(1.0 / np.sqrt(DK))
    for c in range(NCORES):
        b, hg = c // 2, c % 2
        hs = hg * HC
        m = {
            "qT": np.ascontiguousarray(q[b].T).astype(bf16),
            "kT": np.ascontiguousarray(k[b].T).astype(bf16),
            "vT": np.ascontiguousarray(v[b].T).astype(bf16),
            # head-major col blocks; fold 1/sqrt(dk) into Wq
            "wq": np.ascontiguousarray(
                Wq[hs:hs + HC].transpose(1, 0, 2).reshape(D, HC * DK) * scale
            ).astype(bf16),
            "wk": np.ascontiguousarray(
                Wk[hs:hs + HC].transpose(1, 0, 2).reshape(D, HC * DK)
            ).astype(bf16),
            "wv": np.ascontiguousarray(
                Wv[hs:hs + HC].transpose(1, 0, 2).reshape(D, HC * DV)
            ).astype(bf16),
            "wo": np.ascontiguousarray(Wo[hs * DV:(hs + HC) * DV]).astype(bf16),
            "maskr": np.ascontiguousarray(
                maskf[b].reshape(NKC, 128).T).astype(np.float32),
        }
        in_maps.append(m)
    return in_maps


def kernel(q, k, v, mask, Wq, Wk, Wv, Wo, _trace=False):
    from concourse.bass_utils import run_bass_kernel_spmd

    nc = _get_nc()
    in_maps = _shard_inputs(q, k, v, mask, Wq, Wk, Wv, Wo)
    res = run_bass_kernel_spmd(nc, in_maps, list(range(NCORES)),
                               trace=_trace)
    out = np.zeros((B, S, D), np.float32)
    for c in range(NCORES):
        out[c // 2] += res.results[c]["out"]
    if _trace:
        _COMPILED["last_result"] = res
    return out


# revision 24
# speedup vs baseline: 1.0758x; 1.0758x over previous
"""Multi-head attention (B=4, S=2048, D=1024, H=16, dk=dv=64) on 8 TRN2 cores.

Sharding: core c = 2*b + hg handles batch b = c//2 and heads
[hg*8, hg*8+8). Each core computes a partial output
(its 8 heads' contribution through Wo); the host adds the two partials
per batch.

Per-core pipeline (matmul inputs bf16, PSUM fp32, ScalarE exp paces the
attention phase at ~1.5us per [128,1024] tile):
  Prefix: stage kT/vT with 8 big DMAs each; k-projection with
    c-outer/qb-inner loops (stationary weight reuse, 4 PSUM banks);
    q-projection for qb0. khT[p] stores the head PAIR packed
    (rows 0-63 = h0 dk, 64-127 = h1 dk) - same as qhT.
  Attention per (qb, pair): per key chunk kc, two K=64 scores matmuls
    (tile_position auto (0,0)/(64,0) - concurrent row tiles on HW) into
    one [128,1024] PSUM tile (h0 cols 0:512, h1 512:1024); one exp ACT
    per tile; two mix matmuls (lhsT = vh_aug [128 keys, 65] with a
    mask/ones sums column) accumulating into mixP/mixR.
  Normalize per pair: DVE casts PSUM sums rows (row 64) to fp16 at
    partition 64, one DMA moves both heads' sums to partition 0, two
    K=1 fp16 broadcast matmuls replicate them across 64 partitions,
    DVE reciprocal + multiply produce normalized bf16 mixT; h1 is
    DMA-shifted to partitions 64-127.
  v-projection is interleaved into pair 0's score loop; q-projection
    (qb+1) and Wo (qb-1) groups fill interleave slots in later pairs so
    the PE works inside the ScalarE exp slack.
  Wo: out += normT.T @ Wo accumulated over the 4 pairs; DVE evac; DMA.
"""

import numpy as np

B, S, D = 4, 2048, 1024
H, DK, DV = 16, 64, 64
HC = 8          # heads per core
NP = HC // 2    # head pairs per core
NCORES = 8
NC_CHUNKS = D // 128    # 8 contraction chunks over D
NKC = S // 128          # 16 key chunks
NQB = S // 512          # 4 query blocks
VW = HC * 65            # vh storage: 65 cols per head (dv | mask)

_COMPILED = {}


def _build_nc():
    import concourse.tile as tile
    from concourse import bacc, mybir
    from contextlib import ExitStack

    F32 = mybir.dt.float32
    F16 = mybir.dt.float16
    BF16 = mybir.dt.bfloat16
    EXP = mybir.ActivationFunctionType.Exp

    nc = bacc.Bacc("TRN2", target_bir_lowering=False, debug=False,
                   num_devices=NCORES)

    qT = nc.dram_tensor("qT", [D, S], BF16, kind="ExternalInput").ap()
    kT = nc.dram_tensor("kT", [D, S], BF16, kind="ExternalInput").ap()
    vT = nc.dram_tensor("vT", [D, S], BF16, kind="ExternalInput").ap()
    wq = nc.dram_tensor("wq", [D, HC * DK], BF16, kind="ExternalInput").ap()
    wk = nc.dram_tensor("wk", [D, HC * DK], BF16, kind="ExternalInput").ap()
    wv = nc.dram_tensor("wv", [D, HC * DV], BF16, kind="ExternalInput").ap()
    wo = nc.dram_tensor("wo", [HC * DV, D], BF16, kind="ExternalInput").ap()
    maskr = nc.dram_tensor("maskr", [128, NKC], F32, kind="ExternalInput").ap()
    out = nc.dram_tensor("out", [S, D], F32, kind="ExternalOutput").ap()

    with tile.TileContext(nc) as tc:
        with ExitStack() as ctx:
            const_pool = ctx.enter_context(tc.tile_pool(name="const", bufs=1))
            w_pool = ctx.enter_context(tc.tile_pool(name="weights", bufs=1))
            act_pool = ctx.enter_context(tc.tile_pool(name="acts", bufs=1))

            mask_sb = const_pool.tile([128, NKC], F32)
            ones_sb = const_pool.tile([128, HC], BF16)
            sel16 = const_pool.tile([1, DV], F16)

            # weights staged chunk-major: col block c holds rows c*128..+128
            wq_sb = w_pool.tile([128, NC_CHUNKS * 512], BF16, tag="wq")
            wv_sb = w_pool.tile([128, NC_CHUNKS * 512], BF16, tag="wv")
            wo_sb = w_pool.tile([128, NP * 1024], BF16, tag="wo")

            # persistent activations (pair-packed: rows 0-63 h0, 64-127 h1)
            qhT = [act_pool.tile([128, S], BF16, tag=f"qhT{p}", name=f"qhT{p}")
                   for p in range(NP)]
            khT = [act_pool.tile([128, S], BF16, tag=f"khT{p}", name=f"khT{p}")
                   for p in range(NP)]
            vhs = [act_pool.tile([128, VW], BF16, tag=f"vh{t}", name=f"vh{t}")
                   for t in range(NKC)]

            # attention pools created BEFORE the staging scopes so that the
            # staging scopes can close in LIFO order mid-kernel.
            qst_pool = ctx.enter_context(
                tc.tile_pool(name="qstage", bufs=10))
            sc_pool = ctx.enter_context(
                tc.tile_pool(name="scpsum", bufs=2, space="PSUM"))
            mx_pool = ctx.enter_context(
                tc.tile_pool(name="mxpsum", bufs=2, space="PSUM"))
            sh_pool = ctx.enter_context(
                tc.tile_pool(name="shpsum", bufs=2, space="PSUM"))
            exp_pool = ctx.enter_context(tc.tile_pool(name="exp", bufs=4))
            stg_pool = ctx.enter_context(tc.tile_pool(name="stg", bufs=2))
            norm_pool = ctx.enter_context(tc.tile_pool(name="norm", bufs=9))
            tmp_pool = ctx.enter_context(tc.tile_pool(name="tmp", bufs=2))
            s64_pool = ctx.enter_context(tc.tile_pool(name="s64", bufs=2))
            srow_pool = ctx.enter_context(tc.tile_pool(name="srow", bufs=2))
            out_pool = ctx.enter_context(tc.tile_pool(name="outsb", bufs=2))

            # ---- issue all input DMAs up front (big tiles first) ----
            # vstage allocated first, kstage second: kstage scope closes
            # right after the prefix (LIFO), vstage after pair 0's v-proj.
            vst_ctx = ExitStack()
            vst_pool = vst_ctx.enter_context(
                tc.tile_pool(name="vstage", bufs=1))
            vstg = [vst_pool.tile([128, S], BF16, tag=f"vst{c}",
                                  name=f"vst{c}") for c in range(NC_CHUNKS)]
            kst_ctx = ExitStack()
            kst_pool = kst_ctx.enter_context(
                tc.tile_pool(name="kstage", bufs=1))
            wk_sb = kst_pool.tile([128, NC_CHUNKS * 512], BF16, tag="wk")
            kstg = [kst_pool.tile([128, S], BF16, tag=f"kst{c}",
                                  name=f"kst{c}") for c in range(NC_CHUNKS)]
            for c in range(NC_CHUNKS):
                nc.sync.dma_start(wk_sb[:, c * 512:(c + 1) * 512],
                                  wk[c * 128:(c + 1) * 128, :])
                nc.sync.dma_start(kstg[c][:], kT[c * 128:(c + 1) * 128, :])

            def stage_q(qb):
                stg = []
                for c in range(NC_CHUNKS):
                    t = qst_pool.tile([128, 512], BF16, tag="qst",
                                      name=f"qst{qb}_{c}")
                    nc.sync.dma_start(
                        t[:], qT[c * 128:(c + 1) * 128,
                                 qb * 512:(qb + 1) * 512])
                    stg.append(t)
                return stg

            for c in range(NC_CHUNKS):
                nc.sync.dma_start(wq_sb[:, c * 512:(c + 1) * 512],
                                  wq[c * 128:(c + 1) * 128, :])
            qstg0 = stage_q(0)
            nc.sync.dma_start(mask_sb[:], maskr[:])
            for c in range(NC_CHUNKS):
                nc.sync.dma_start(wv_sb[:, c * 512:(c + 1) * 512],
                                  wv[c * 128:(c + 1) * 128, :])
            for p in range(NP):
                nc.sync.dma_start(wo_sb[:, p * 1024:(p + 1) * 1024],
                                  wo[p * 128:(p + 1) * 128, :])
            for c in range(NC_CHUNKS):
                nc.sync.dma_start(vstg[c][:], vT[c * 128:(c + 1) * 128, :])
            nc.vector.memset(ones_sb[:], 1.0)
            nc.vector.memset(sel16[:], 1.0)

            # ---- prefix: k projection (c-outer, stationary reuse) ----
            # kps tiles come from the attention sc pool ([128,1024] holds
            # two query blocks side by side).
            for p in range(NP):
                kpsA = sc_pool.tile([128, 1024], F32, tag="sc",
                                    name=f"kpsA{p}")
                kpsB = sc_pool.tile([128, 1024], F32, tag="sc",
                                    name=f"kpsB{p}")
                halves = [kpsA[:, 0:512], kpsA[:, 512:1024],
                          kpsB[:, 0:512], kpsB[:, 512:1024]]
                for c in range(NC_CHUNKS):
                    wsl = wk_sb[:, c * 512 + p * 128:
                                c * 512 + (p + 1) * 128]
                    for qb in range(NQB):
                        nc.tensor.matmul(
                            halves[qb],
                            lhsT=wsl,
                            rhs=kstg[c][:, qb * 512:(qb + 1) * 512],
                            start=(c == 0), stop=(c == NC_CHUNKS - 1))
                nc.vector.tensor_copy(khT[p][:, 0:1024], kpsA[:])
                nc.vector.tensor_copy(khT[p][:, 1024:2048], kpsB[:])
            # q projection for qb0
            for p in range(NP):
                qps = sh_pool.tile([128, 512], F32, tag="sh",
                                   name=f"qps0_{p}")
                for c in range(NC_CHUNKS):
                    nc.tensor.matmul(
                        qps[:],
                        lhsT=wq_sb[:, c * 512 + p * 128:
                                   c * 512 + (p + 1) * 128],
                        rhs=qstg0[c][:],
                        start=(c == 0), stop=(c == NC_CHUNKS - 1))
                nc.vector.tensor_copy(qhT[p][:, 0:512], qps[:])
            kst_ctx.close()   # free kT staging (32KB/partition)

            def vproj_chunk(t):
                """Project v for token chunk t into vhs[t] (all 8 heads)."""
                vps = sh_pool.tile([128, 512], F32, tag="sh")
                for c in range(NC_CHUNKS):
                    nc.tensor.matmul(
                        vps[:],
                        lhsT=vstg[c][:, t * 128:(t + 1) * 128],
                        rhs=wv_sb[:, c * 512:(c + 1) * 512],
                        start=(c == 0), stop=(c == NC_CHUNKS - 1))
                dst_dv = vhs[t][:, 0:VW].rearrange(
                    "p (h x) -> p h x", x=65)[:, :, 0:DV]
                src_dv = vps[:].rearrange("p (h x) -> p h x", x=DV)
                nc.vector.tensor_scalar_mul(dst_dv, src_dv,
                                            mask_sb[:, t:t + 1])
                dst_m = vhs[t][:, 0:VW].rearrange(
                    "p (h x) -> p h x", x=65)[:, :, DV:DV + 1]
                src_m = ones_sb[:, 0:HC].rearrange("p (h x) -> p h x", x=1)
                nc.vector.tensor_scalar_mul(dst_m, src_m,
                                            mask_sb[:, t:t + 1])

            def qproj_group(qb, p, stg):
                """Project q for (qb, pair p) into qhT[p]."""
                qps = sh_pool.tile([128, 512], F32, tag="sh")
                for c in range(NC_CHUNKS):
                    nc.tensor.matmul(
                        qps[:],
                        lhsT=wq_sb[:, c * 512 + p * 128:
                                   c * 512 + (p + 1) * 128],
                        rhs=stg[c][:],
                        start=(c == 0), stop=(c == NC_CHUNKS - 1))
                nc.vector.tensor_copy(qhT[p][:, qb * 512:(qb + 1) * 512],
                                      qps[:])

            def wo_group(qb, tt, dh, normT):
                """One Wo output tile [128 q, 512 d] accumulated over pairs."""
                wps = sh_pool.tile([128, 512], F32, tag="sh")
                for p in range(NP):
                    nc.tensor.matmul(
                        wps[:],
                        lhsT=normT[p][:, tt * 128:(tt + 1) * 128],
                        rhs=wo_sb[:, p * 1024 + dh * 512:
                                  p * 1024 + (dh + 1) * 512],
                        start=(p == 0), stop=(p == NP - 1))
                osb = out_pool.tile([128, 512], F32, tag="osb")
                if qb == NQB - 1:
                    nc.scalar.copy(osb[:], wps[:])
                else:
                    nc.vector.tensor_copy(osb[:], wps[:])
                nc.sync.dma_start(
                    out[qb * 512 + tt * 128:qb * 512 + (tt + 1) * 128,
                        dh * 512:(dh + 1) * 512], osb[:])

            # interleave slot queue: list of thunks
            pending = []
            post_norm = []

            def run_slot():
                if pending:
                    pending.pop(0)()

            for qb in range(NQB):
                normT = []
                for p in range(NP):
                    first = (qb == 0 and p == 0)
                    last = (qb == NQB - 1 and p == NP - 1)
                    # split pairs: odd key chunks exp'd in SBUF mega-ACTs
                    # (DVE casts PSUM->SBUF bf16; one in-place 4096-el ACT
                    # per 4 odd chunks) to offload the ScalarE pace-setter
                    use_split = not first and not last
                    lag = 4 if first else 2
                    if p == NP - 1 and qb + 1 < NQB:
                        # queue q-projection for qb+1 into this pair's slots
                        stg = stage_q(qb + 1)
                        for pp in range(NP):
                            pending.append(
                                lambda qb=qb, pp=pp, stg=stg: qproj_group(
                                    qb + 1, pp, stg))
                    h0, h1 = 2 * p, 2 * p + 1
                    qful = qhT[p][:, qb * 512:(qb + 1) * 512]
                    mixP = mx_pool.tile([128, 512], F32, tag="mx")
                    mixR = mx_pool.tile([128, 512], F32, tag="mx")
                    exps = []

                    def mix_step(lk):
                        for h, mx in ((h0, mixP), (h1, mixR)):
                            nc.tensor.matmul(
                                mx[0:65, :],
                                lhsT=vhs[lk][:, h * 65:h * 65 + 65],
                                rhs=exps[lk][:, (h % 2) * 512:
                                             (h % 2) * 512 + 512],
                                start=(lk == 0), stop=(lk == NKC - 1))

                    # scores + exp per key chunk (two K=64 row tiles)
                    for kc in range(NKC):
                        ksl = slice(kc * 128, (kc + 1) * 128)
                        sc = sc_pool.tile([128, 1024], F32, tag="sc")
                        nc.tensor.matmul(
                            sc[:, 0:512],
                            lhsT=khT[p][0:64, ksl], rhs=qful[0:64, :],
                            start=True, stop=True)
                        nc.tensor.matmul(
                            sc[:, 512:1024],
                            lhsT=khT[p][64:128, ksl], rhs=qful[64:128, :],
                            start=True, stop=True)
                        if use_split and kc % 2 == 1 and kc < 15:
                            j = kc // 2
                            if j % 4 == 0:
                                stg_cur = stg_pool.tile([128, 4096], BF16,
                                                        tag="stg",
                                                        name="stg")
                            ssl = slice((j % 4) * 1024, (j % 4) * 1024 + 1024)
                            nc.vector.tensor_copy(stg_cur[:, ssl], sc[:])
                            exps.append(stg_cur[:, ssl])
                            if j == 3:
                                nc.scalar.activation(stg_cur[:], stg_cur[:],
                                                     EXP)
                            elif j == 6:
                                # batch B covers odds 9,11,13 only so the
                                # drain never waits on a late mega-ACT
                                nc.scalar.activation(stg_cur[:, 0:3072],
                                                     stg_cur[:, 0:3072], EXP)
                        else:
                            ex = exp_pool.tile([128, 1024], BF16, tag="exp")
                            nc.scalar.activation(ex[:], sc[:], EXP)
                            exps.append(ex)
                        if first:
                            # v projection rides pair 0's score loop
                            vproj_chunk(kc)
                        if kc == 5 and post_norm:
                            post_norm.pop(0)()
                        if use_split:
                            if kc >= 2 and kc % 2 == 0:
                                mix_step(kc - 2)
                            if 9 <= kc <= 12:
                                mix_step(2 * (kc - 9) + 1)
                        elif kc >= lag:
                            mix_step(kc - lag)
                        if (not first and kc % 4 == 3
                                and not (p == 0 and kc == 3)):
                            run_slot()
                    if use_split:
                        for lk in (14, 9, 11, 13, 15):
                            mix_step(lk)
                    else:
                        for lk in range(NKC - lag, NKC):
                            mix_step(lk)
                    if first:
                        vst_ctx.close()   # free vT staging
                    # ---- normalize (pre): sums rows -> fp16 -> partition 0
                    s64 = s64_pool.tile([128, 1024], F16, tag="s64")
                    nc.vector.tensor_copy(s64[64:65, 0:512], mixP[64:65, :])
                    nc.vector.tensor_copy(s64[64:65, 512:1024],
                                          mixR[64:65, :])
                    srow = srow_pool.tile([1, 1024], F16, tag="srow")
                    nc.gpsimd.dma_start(srow[0:1, :], s64[64:65, :])

                    def norm_post(mixP=mixP, mixR=mixR, srow=srow,
                                  normT=normT):
                        # broadcast (K=1 fp16 matmuls), recip, multiply;
                        # deferred into the next pair so the PE never waits
                        # on the sums DMA.
                        bc0 = sh_pool.tile([128, 512], F32, tag="sh")
                        bc1 = sh_pool.tile([128, 512], F32, tag="sh")
                        nc.tensor.matmul(bc0[0:DV, :], lhsT=sel16[:],
                                         rhs=srow[0:1, 0:512],
                                         start=True, stop=True)
                        nc.tensor.matmul(bc1[0:DV, :], lhsT=sel16[:],
                                         rhs=srow[0:1, 512:1024],
                                         start=True, stop=True)
                        rec0 = tmp_pool.tile([64, 512], F32, tag="rec")
                        rec1 = tmp_pool.tile([64, 512], F32, tag="rec")
                        nc.vector.reciprocal_approx_fast(rec0[:],
                                                         bc0[0:64, :])
                        nc.vector.reciprocal_approx_fast(rec1[:],
                                                         bc1[0:64, :])
                        nt = norm_pool.tile([128, 512], BF16, tag="norm")
                        normT.append(nt)
                        nc.vector.tensor_mul(nt[0:64, :], mixP[0:64, :],
                                             rec0[:])
                        sh1 = tmp_pool.tile([64, 512], BF16, tag="sh1")
                        nc.vector.tensor_mul(sh1[:], mixR[0:64, :], rec1[:])
                        nc.gpsimd.dma_start(nt[64:128, :], sh1[:])

                    if qb == NQB - 1 and p == NP - 1:
                        norm_post()
                    else:
                        post_norm.append(norm_post)

                # queue Wo for this qb into the next qb's interleave slots
                # (normT is filled lazily by deferred norm_post thunks; pass
                # the live list, complete by the time any wo_group runs)
                nt_list = normT
                for tt in range(4):
                    for dh in range(2):
                        pending.append(
                            lambda qb=qb, tt=tt, dh=dh, nt=nt_list: wo_group(
                                qb, tt, dh, nt))
                # last qb: drain all pending now
                if qb == NQB - 1:
                    while pending:
                        run_slot()

    nc.compile()
    return nc


def _get_nc():
    if "nc" not in _COMPILED:
        _COMPILED["nc"] = _build_nc()
    return _COMPILED["nc"]


def _shard_inputs(q, k, v, mask, Wq, Wk, Wv, Wo):
    """Build the per-core input maps (host-side layout prep)."""
    import ml_dtypes

    bf16 = ml_dtypes.bfloat16
    in_maps = []
    maskf = np.asarray(mask).astype(np.float32)
    q = np.asarray(q, np.float32)
    k = np.asarray(k, np.float32)
    v = np.asarray(v, np.float32)
    Wq = np.asarray(Wq, np.float32)
    Wk = np.asarray(Wk, np.float32)
    Wv = np.asarray(Wv, np.float32)
    Wo = np.asarray(Wo, np.float32)
    scale = np.float32(1.0 / np.sqrt(DK))
    for c in range(NCORES):
        b, hg = c // 2, c % 2
        hs = hg * HC
        m = {
            "qT": np.ascontiguousarray(q[b].T).astype(bf16),
            "kT": np.ascontiguousarray(k[b].T).astype(bf16),
            "vT": np.ascontiguousarray(v[b].T).astype(bf16),
            # head-major col blocks; fold 1/sqrt(dk) into Wq
            "wq": np.ascontiguousarray(
                Wq[hs:hs + HC].transpose(1, 0, 2).reshape(D, HC * DK) * scale
            ).astype(bf16),
            "wk": np.ascontiguousarray(
                Wk[hs:hs + HC].transpose(1, 0, 2).reshape(D, HC * DK)
            ).astype(bf16),
            "wv": np.ascontiguousarray(
                Wv[hs:hs + HC].transpose(1, 0, 2).reshape(D, HC * DV)
            ).astype(bf16),
            "wo": np.ascontiguousarray(Wo[hs * DV:(hs + HC) * DV]).astype(bf16),
            "maskr": np.ascontiguousarray(
                maskf[b].reshape(NKC, 128).T).astype(np.float32),
        }
        in_maps.append(m)
    return in_maps


def kernel(q, k, v, mask, Wq, Wk, Wv, Wo, _trace=False):
    from concourse.bass_utils import run_bass_kernel_spmd

    nc = _get_nc()
    in_maps = _shard_inputs(q, k, v, mask, Wq, Wk, Wv, Wo)
    res = run_bass_kernel_spmd(nc, in_maps, list(range(NCORES)),
                               trace=_trace)
    out = np.zeros((B, S, D), np.float32)
    for c in range(NCORES):
        out[c // 2] += res.results[c]["out"]
    if _trace:
        _COMPILED["last_result"] = res
    return out


# revision 25
# speedup vs baseline: 1.2091x; 1.1239x over previous
"""Multi-head attention (B=4, S=2048, D=1024, H=16, dk=dv=64) on 8 TRN2 cores.

Sharding: core c = 2*b + hg handles batch b = c//2 and heads
[hg*8, hg*8+8). Each core computes a partial output
(its 8 heads' contribution through Wo); the host adds the two partials
per batch.

Per-core pipeline (matmul inputs bf16, PSUM fp32, ScalarE exp paces the
attention phase at ~1.5us per [128,1024] tile):
  Prefix: stage kT/vT with 8 big DMAs each; k-projection with
    c-outer/qb-inner loops (stationary weight reuse, 4 PSUM banks);
    q-projection for qb0. khT[p] stores the head PAIR packed
    (rows 0-63 = h0 dk, 64-127 = h1 dk) - same as qhT.
  Attention per (qb, pair): per key chunk kc, two K=64 scores matmuls
    (tile_position auto (0,0)/(64,0) - concurrent row tiles on HW) into
    one [128,1024] PSUM tile (h0 cols 0:512, h1 512:1024); one exp ACT
    per tile; two mix matmuls (lhsT = vh_aug [128 keys, 65] with a
    mask/ones sums column) accumulating into mixP/mixR.
  Normalize per pair: DVE casts PSUM sums rows (row 64) to fp16 at
    partition 64, one DMA moves both heads' sums to partition 0, two
    K=1 fp16 broadcast matmuls replicate them across 64 partitions,
    DVE reciprocal + multiply produce normalized bf16 mixT; h1 is
    DMA-shifted to partitions 64-127.
  v-projection is interleaved into pair 0's score loop; q-projection
    (qb+1) and Wo (qb-1) groups fill interleave slots in later pairs so
    the PE works inside the ScalarE exp slack.
  Wo: out += normT.T @ Wo accumulated over the 4 pairs; DVE evac; DMA.
"""

import numpy as np

B, S, D = 4, 2048, 1024
H, DK, DV = 16, 64, 64
HC = 8          # heads per core
NP = HC // 2    # head pairs per core
NCORES = 8
NC_CHUNKS = D // 128    # 8 contraction chunks over D
NKC = S // 128          # 16 key chunks
NQB = S // 512          # 4 query blocks
VW = HC * 65            # vh storage: 65 cols per head (dv | mask)

_COMPILED = {}


def _build_nc():
    import concourse.tile as tile
    from concourse import bacc, mybir
    from contextlib import ExitStack

    F32 = mybir.dt.float32
    F16 = mybir.dt.float16
    BF16 = mybir.dt.bfloat16
    EXP = mybir.ActivationFunctionType.Exp

    nc = bacc.Bacc("TRN2", target_bir_lowering=False, debug=False,
                   num_devices=NCORES)

    qT = nc.dram_tensor("qT", [D, S], BF16, kind="ExternalInput").ap()
    kT = nc.dram_tensor("kT", [D, S], BF16, kind="ExternalInput").ap()
    vT = nc.dram_tensor("vT", [D, S], BF16, kind="ExternalInput").ap()
    wq = nc.dram_tensor("wq", [D, HC * DK], BF16, kind="ExternalInput").ap()
    wk = nc.dram_tensor("wk", [D, HC * DK], BF16, kind="ExternalInput").ap()
    wv = nc.dram_tensor("wv", [D, HC * DV], BF16, kind="ExternalInput").ap()
    wo = nc.dram_tensor("wo", [HC * DV, D], BF16, kind="ExternalInput").ap()
    maskr = nc.dram_tensor("maskr", [128, NKC], F32, kind="ExternalInput").ap()
    out = nc.dram_tensor("out", [S, D], F32, kind="ExternalOutput").ap()

    with tile.TileContext(nc) as tc:
        with ExitStack() as ctx:
            const_pool = ctx.enter_context(tc.tile_pool(name="const", bufs=1))
            w_pool = ctx.enter_context(tc.tile_pool(name="weights", bufs=1))
            act_pool = ctx.enter_context(tc.tile_pool(name="acts", bufs=1))

            mask_sb = const_pool.tile([128, NKC], F32)
            ones_sb = const_pool.tile([128, HC], BF16)
            sel16 = const_pool.tile([1, DV], F16)

            # weights staged chunk-major: col block c holds rows c*128..+128
            wq_sb = w_pool.tile([128, NC_CHUNKS * 512], BF16, tag="wq")
            wv_sb = w_pool.tile([128, NC_CHUNKS * 512], BF16, tag="wv")
            wo_sb = w_pool.tile([128, NP * 1024], BF16, tag="wo")

            # persistent activations (pair-packed: rows 0-63 h0, 64-127 h1)
            qhT = [act_pool.tile([128, S], BF16, tag=f"qhT{p}", name=f"qhT{p}")
                   for p in range(NP)]
            khT = [act_pool.tile([128, S], BF16, tag=f"khT{p}", name=f"khT{p}")
                   for p in range(NP)]
            vhs = [act_pool.tile([128, VW], BF16, tag=f"vh{t}", name=f"vh{t}")
                   for t in range(NKC)]

            # attention pools created BEFORE the staging scopes so that the
            # staging scopes can close in LIFO order mid-kernel.
            qst_pool = ctx.enter_context(
                tc.tile_pool(name="qstage", bufs=12))
            sc_pool = ctx.enter_context(
                tc.tile_pool(name="scpsum", bufs=2, space="PSUM"))
            mx_pool = ctx.enter_context(
                tc.tile_pool(name="mxpsum", bufs=2, space="PSUM"))
            sh_pool = ctx.enter_context(
                tc.tile_pool(name="shpsum", bufs=2, space="PSUM"))
            exp_pool = ctx.enter_context(tc.tile_pool(name="exp", bufs=6))
            norm_pool = ctx.enter_context(tc.tile_pool(name="norm", bufs=9))
            tmp_pool = ctx.enter_context(tc.tile_pool(name="tmp", bufs=3))
            s64_pool = ctx.enter_context(tc.tile_pool(name="s64", bufs=2))
            srow_pool = ctx.enter_context(tc.tile_pool(name="srow", bufs=2))
            out_pool = ctx.enter_context(tc.tile_pool(name="outsb", bufs=2))

            # ---- issue all input DMAs up front (big tiles first) ----
            # vstage allocated first, kstage second: kstage scope closes
            # right after the prefix (LIFO), vstage after pair 0's v-proj.
            vst_ctx = ExitStack()
            vst_pool = vst_ctx.enter_context(
                tc.tile_pool(name="vstage", bufs=1))
            vstg = [vst_pool.tile([128, S], BF16, tag=f"vst{c}",
                                  name=f"vst{c}") for c in range(NC_CHUNKS)]
            kst_ctx = ExitStack()
            kst_pool = kst_ctx.enter_context(
                tc.tile_pool(name="kstage", bufs=1))
            wk_sb = kst_pool.tile([128, NC_CHUNKS * 512], BF16, tag="wk")
            kstg = [kst_pool.tile([128, S], BF16, tag=f"kst{c}",
                                  name=f"kst{c}") for c in range(NC_CHUNKS)]
            for c in range(NC_CHUNKS):
                nc.sync.dma_start(wk_sb[:, c * 512:(c + 1) * 512],
                                  wk[c * 128:(c + 1) * 128, :])
                nc.sync.dma_start(kstg[c][:], kT[c * 128:(c + 1) * 128, :])

            def stage_q(qb):
                stg = []
                for c in range(NC_CHUNKS):
                    t = qst_pool.tile([128, 512], BF16, tag="qst",
                                      name=f"qst{qb}_{c}")
                    nc.sync.dma_start(
                        t[:], qT[c * 128:(c + 1) * 128,
                                 qb * 512:(qb + 1) * 512])
                    stg.append(t)
                return stg

            for c in range(NC_CHUNKS):
                nc.sync.dma_start(wq_sb[:, c * 512:(c + 1) * 512],
                                  wq[c * 128:(c + 1) * 128, :])
            qstg0 = stage_q(0)
            nc.sync.dma_start(mask_sb[:], maskr[:])
            for c in range(NC_CHUNKS):
                nc.sync.dma_start(wv_sb[:, c * 512:(c + 1) * 512],
                                  wv[c * 128:(c + 1) * 128, :])
            for p in range(NP):
                nc.sync.dma_start(wo_sb[:, p * 1024:(p + 1) * 1024],
                                  wo[p * 128:(p + 1) * 128, :])
            for c in range(NC_CHUNKS):
                nc.sync.dma_start(vstg[c][:], vT[c * 128:(c + 1) * 128, :])
            nc.vector.memset(ones_sb[:], 1.0)
            nc.vector.memset(sel16[:], 1.0)

            # ---- prefix: k projection (c-outer, stationary reuse) ----
            # kps tiles come from the attention sc pool ([128,1024] holds
            # two query blocks side by side).
            for p in range(NP):
                kpsA = sc_pool.tile([128, 1024], F32, tag="sc",
                                    name=f"kpsA{p}")
                kpsB = sc_pool.tile([128, 1024], F32, tag="sc",
                                    name=f"kpsB{p}")
                halves = [kpsA[:, 0:512], kpsA[:, 512:1024],
                          kpsB[:, 0:512], kpsB[:, 512:1024]]
                for c in range(NC_CHUNKS):
                    wsl = wk_sb[:, c * 512 + p * 128:
                                c * 512 + (p + 1) * 128]
                    for qb in range(NQB):
                        nc.tensor.matmul(
                            halves[qb],
                            lhsT=wsl,
                            rhs=kstg[c][:, qb * 512:(qb + 1) * 512],
                            start=(c == 0), stop=(c == NC_CHUNKS - 1))
                nc.vector.tensor_copy(khT[p][:, 0:1024], kpsA[:])
                nc.vector.tensor_copy(khT[p][:, 1024:2048], kpsB[:])
            # q projection for qb0
            for p in range(NP):
                qps = sh_pool.tile([128, 512], F32, tag="sh",
                                   name=f"qps0_{p}")
                for c in range(NC_CHUNKS):
                    nc.tensor.matmul(
                        qps[:],
                        lhsT=wq_sb[:, c * 512 + p * 128:
                                   c * 512 + (p + 1) * 128],
                        rhs=qstg0[c][:],
                        start=(c == 0), stop=(c == NC_CHUNKS - 1))
                nc.vector.tensor_copy(qhT[p][:, 0:512], qps[:])
            kst_ctx.close()   # free kT staging (32KB/partition)

            def vproj_chunk(t):
                """Project v for token chunk t into vhs[t] (all 8 heads)."""
                vps = sh_pool.tile([128, 512], F32, tag="sh")
                for c in range(NC_CHUNKS):
                    nc.tensor.matmul(
                        vps[:],
                        lhsT=vstg[c][:, t * 128:(t + 1) * 128],
                        rhs=wv_sb[:, c * 512:(c + 1) * 512],
                        start=(c == 0), stop=(c == NC_CHUNKS - 1))
                dst_dv = vhs[t][:, 0:VW].rearrange(
                    "p (h x) -> p h x", x=65)[:, :, 0:DV]
                src_dv = vps[:].rearrange("p (h x) -> p h x", x=DV)
                nc.vector.tensor_scalar_mul(dst_dv, src_dv,
                                            mask_sb[:, t:t + 1])
                dst_m = vhs[t][:, 0:VW].rearrange(
                    "p (h x) -> p h x", x=65)[:, :, DV:DV + 1]
                src_m = ones_sb[:, 0:HC].rearrange("p (h x) -> p h x", x=1)
                nc.vector.tensor_scalar_mul(dst_m, src_m,
                                            mask_sb[:, t:t + 1])

            def qproj_group(qb, p, stg):
                """Project q for (qb, pair p) into qhT[p]."""
                qps = sh_pool.tile([128, 512], F32, tag="sh")
                for c in range(NC_CHUNKS):
                    nc.tensor.matmul(
                        qps[:],
                        lhsT=wq_sb[:, c * 512 + p * 128:
                                   c * 512 + (p + 1) * 128],
                        rhs=stg[c][:],
                        start=(c == 0), stop=(c == NC_CHUNKS - 1))
                nc.vector.tensor_copy(qhT[p][:, qb * 512:(qb + 1) * 512],
                                      qps[:])

            def wo_group(qb, tt, dh, normT):
                """One Wo output tile [128 q, 512 d] accumulated over pairs."""
                wps = sh_pool.tile([128, 512], F32, tag="sh")
                for p in range(NP):
                    nc.tensor.matmul(
                        wps[:],
                        lhsT=normT[p][:, tt * 128:(tt + 1) * 128],
                        rhs=wo_sb[:, p * 1024 + dh * 512:
                                  p * 1024 + (dh + 1) * 512],
                        start=(p == 0), stop=(p == NP - 1))
                osb = out_pool.tile([128, 512], F32, tag="osb")
                if qb == NQB - 1:
                    nc.scalar.copy(osb[:], wps[:])
                else:
                    nc.vector.tensor_copy(osb[:], wps[:])
                nc.sync.dma_start(
                    out[qb * 512 + tt * 128:qb * 512 + (tt + 1) * 128,
                        dh * 512:(dh + 1) * 512], osb[:])

            # interleave slot queue: list of thunks
            pending = []
            post_norm = []

            def run_slot():
                if pending:
                    pending.pop(0)()

            for qb in range(NQB):
                normT = []
                for p in range(NP):
                    first = (qb == 0 and p == 0)
                    lag = 4 if first else 2
                    if p == NP - 1 and qb + 1 < NQB:
                        # queue q-projection for qb+1 into this pair's slots
                        stg = stage_q(qb + 1)
                        for pp in range(NP):
                            pending.append(
                                lambda qb=qb, pp=pp, stg=stg: qproj_group(
                                    qb + 1, pp, stg))
                    h0, h1 = 2 * p, 2 * p + 1
                    qful = qhT[p][:, qb * 512:(qb + 1) * 512]
                    mixP = mx_pool.tile([128, 512], F32, tag="mx")
                    mixR = mx_pool.tile([128, 512], F32, tag="mx")
                    exps = []

                    def mix_step(lk):
                        for h, mx in ((h0, mixP), (h1, mixR)):
                            nc.tensor.matmul(
                                mx[0:65, :],
                                lhsT=vhs[lk][:, h * 65:h * 65 + 65],
                                rhs=exps[lk][:, (h % 2) * 512:
                                             (h % 2) * 512 + 512],
                                start=(lk == 0), stop=(lk == NKC - 1))

                    # scores + exp per key chunk (two K=64 row tiles)
                    for kc in range(NKC):
                        ksl = slice(kc * 128, (kc + 1) * 128)
                        sc = sc_pool.tile([128, 1024], F32, tag="sc")
                        nc.tensor.matmul(
                            sc[:, 0:512],
                            lhsT=khT[p][0:64, ksl], rhs=qful[0:64, :],
                            start=True, stop=True)
                        nc.tensor.matmul(
                            sc[:, 512:1024],
                            lhsT=khT[p][64:128, ksl], rhs=qful[64:128, :],
                            start=True, stop=True)
                        ex = exp_pool.tile([128, 1024], BF16, tag="exp")
                        nc.scalar.activation(ex[:], sc[:], EXP)
                        exps.append(ex)
                        if first:
                            # v projection rides pair 0's score loop
                            vproj_chunk(kc)
                        if kc == 5 and post_norm:
                            post_norm.pop(0)()
                        if kc >= lag:
                            mix_step(kc - lag)
                        if (not first and kc % 4 == 3
                                and not (p == 0 and kc == 3)):
                            run_slot()
                    for lk in range(NKC - lag, NKC):
                        mix_step(lk)
                    if first:
                        vst_ctx.close()   # free vT staging
                    # ---- normalize (pre): sums rows -> fp16 -> partition 0
                    s64 = s64_pool.tile([128, 1024], F16, tag="s64")
                    nc.vector.tensor_copy(s64[64:65, 0:512], mixP[64:65, :])
                    nc.vector.tensor_copy(s64[64:65, 512:1024],
                                          mixR[64:65, :])
                    srow = srow_pool.tile([1, 1024], F16, tag="srow")
                    nc.gpsimd.dma_start(srow[0:1, :], s64[64:65, :])

                    def norm_post(mixP=mixP, mixR=mixR, srow=srow,
                                  normT=normT):
                        # broadcast (K=1 fp16 matmuls), recip, multiply;
                        # deferred into the next pair so the PE never waits
                        # on the sums DMA.
                        bc0 = sh_pool.tile([128, 512], F32, tag="sh")
                        bc1 = sh_pool.tile([128, 512], F32, tag="sh")
                        nc.tensor.matmul(bc0[0:DV, :], lhsT=sel16[:],
                                         rhs=srow[0:1, 0:512],
                                         start=True, stop=True)
                        nc.tensor.matmul(bc1[0:DV, :], lhsT=sel16[:],
                                         rhs=srow[0:1, 512:1024],
                                         start=True, stop=True)
                        rec0 = tmp_pool.tile([64, 512], F32, tag="rec")
                        rec1 = tmp_pool.tile([64, 512], F32, tag="rec")
                        nc.vector.reciprocal_approx_fast(rec0[:],
                                                         bc0[0:64, :])
                        nc.vector.reciprocal_approx_fast(rec1[:],
                                                         bc1[0:64, :])
                        nt = norm_pool.tile([128, 512], BF16, tag="norm")
                        normT.append(nt)
                        nc.vector.tensor_mul(nt[0:64, :], mixP[0:64, :],
                                             rec0[:])
                        sh1 = tmp_pool.tile([64, 512], BF16, tag="sh1")
                        nc.vector.tensor_mul(sh1[:], mixR[0:64, :], rec1[:])
                        nc.gpsimd.dma_start(nt[64:128, :], sh1[:])

                    if qb == NQB - 1 and p == NP - 1:
                        norm_post()
                    else:
                        post_norm.append(norm_post)

                # queue Wo for this qb into the next qb's interleave slots
                # (normT is filled lazily by deferred norm_post thunks; pass
                # the live list, complete by the time any wo_group runs)
                nt_list = normT
                for tt in range(4):
                    for dh in range(2):
                        pending.append(
                            lambda qb=qb, tt=tt, dh=dh, nt=nt_list: wo_group(
                                qb, tt, dh, nt))
                # last qb: drain all pending now
                if qb == NQB - 1:
                    while pending:
                        run_slot()

    nc.compile()
    return nc


def _get_nc():
    if "nc" not in _COMPILED:
        _COMPILED["nc"] = _build_nc()
    return _COMPILED["nc"]


def _shard_inputs(q, k, v, mask, Wq, Wk, Wv, Wo):
    """Build the per-core input maps (host-side layout prep)."""
    import ml_dtypes

    bf16 = ml_dtypes.bfloat16
    in_maps = []
    maskf = np.asarray(mask).astype(np.float32)
    q = np.asarray(q, np.float32)
    k = np.asarray(k, np.float32)
    v = np.asarray(v, np.float32)
    Wq = np.asarray(Wq, np.float32)
    Wk = np.asarray(Wk, np.float32)
    Wv = np.asarray(Wv, np.float32)
    Wo = np.asarray(Wo, np.float32)
    scale = np.float32(1.0 / np.sqrt(DK))
    for c in range(NCORES):
        b, hg = c // 2, c % 2
        hs = hg * HC
        m = {
            "qT": np.ascontiguousarray(q[b].T).astype(bf16),
            "kT": np.ascontiguousarray(k[b].T).astype(bf16),
            "vT": np.ascontiguousarray(v[b].T).astype(bf16),
            # head-major col blocks; fold 1/sqrt(dk) into Wq
            "wq": np.ascontiguousarray(
                Wq[hs:hs + HC].transpose(1, 0, 2).reshape(D, HC * DK) * scale
            ).astype(bf16),
            "wk": np.ascontiguousarray(
                Wk[hs:hs + HC].transpose(1, 0, 2).reshape(D, HC * DK)
            ).astype(bf16),
            "wv": np.ascontiguousarray(
                Wv[hs:hs + HC].transpose(1, 0, 2).reshape(D, HC * DV)
            ).astype(bf16),
            "wo": np.ascontiguousarray(Wo[hs * DV:(hs + HC) * DV]).astype(bf16),
            "maskr": np.ascontiguousarray(
                maskf[b].reshape(NKC, 128).T).astype(np.float32),
        }
        in_maps.append(m)
    return in_maps


def kernel(q, k, v, mask, Wq, Wk, Wv, Wo, _trace=False):
    from concourse.bass_utils import run_bass_kernel_spmd

    nc = _get_nc()
    in_maps = _shard_inputs(q, k, v, mask, Wq, Wk, Wv, Wo)
    res = run_bass_kernel_spmd(nc, in_maps, list(range(NCORES)),
                               trace=_trace)
    out = np.zeros((B, S, D), np.float32)
    for c in range(NCORES):
        out[c // 2] += res.results[c]["out"]
    if _trace:
        _COMPILED["last_result"] = res
    return out


# revision 26
# speedup vs baseline: 1.2222x; 1.0109x over previous
"""Multi-head attention (B=4, S=2048, D=1024, H=16, dk=dv=64) on 8 TRN2 cores.

Sharding: core c = 2*b + hg handles batch b = c//2 and heads
[hg*8, hg*8+8). Each core computes a partial output
(its 8 heads' contribution through Wo); the host adds the two partials
per batch.

Per-core pipeline (matmul inputs bf16, PSUM fp32, ScalarE exp paces the
attention phase at ~1.5us per [128,1024] tile):
  Prefix: stage kT/vT with 8 big DMAs each; k-projection with
    c-outer/qb-inner loops (stationary weight reuse, 4 PSUM banks);
    q-projection for qb0. khT[p] stores the head PAIR packed
    (rows 0-63 = h0 dk, 64-127 = h1 dk) - same as qhT.
  Attention per (qb, pair): per key chunk kc, two K=64 scores matmuls
    (tile_position auto (0,0)/(64,0) - concurrent row tiles on HW) into
    one [128,1024] PSUM tile (h0 cols 0:512, h1 512:1024); one exp ACT
    per tile; two mix matmuls (lhsT = vh_aug [128 keys, 65] with a
    mask/ones sums column) accumulating into mixP/mixR.
  Normalize per pair: DVE casts PSUM sums rows (row 64) to fp16 at
    partition 64, one DMA moves both heads' sums to partition 0, two
    K=1 fp16 broadcast matmuls replicate them across 64 partitions,
    DVE reciprocal + multiply produce normalized bf16 mixT; h1 is
    DMA-shifted to partitions 64-127.
  v-projection is interleaved into pair 0's score loop; q-projection
    (qb+1) and Wo (qb-1) groups fill interleave slots in later pairs so
    the PE works inside the ScalarE exp slack.
  Wo: out += normT.T @ Wo accumulated over the 4 pairs; DVE evac; DMA.
"""

import numpy as np

B, S, D = 4, 2048, 1024
H, DK, DV = 16, 64, 64
HC = 8          # heads per core
NP = HC // 2    # head pairs per core
NCORES = 8
NC_CHUNKS = D // 128    # 8 contraction chunks over D
NKC = S // 128          # 16 key chunks
NQB = S // 512          # 4 query blocks
VW = HC * 65            # vh storage: 65 cols per head (dv | mask)

_COMPILED = {}


def _build_nc():
    import concourse.tile as tile
    from concourse import bacc, mybir
    from contextlib import ExitStack

    F32 = mybir.dt.float32
    F16 = mybir.dt.float16
    BF16 = mybir.dt.bfloat16
    EXP = mybir.ActivationFunctionType.Exp

    nc = bacc.Bacc("TRN2", target_bir_lowering=False, debug=False,
                   num_devices=NCORES)

    qT = nc.dram_tensor("qT", [D, S], BF16, kind="ExternalInput").ap()
    kT = nc.dram_tensor("kT", [D, S], BF16, kind="ExternalInput").ap()
    vT = nc.dram_tensor("vT", [D, S], BF16, kind="ExternalInput").ap()
    wq = nc.dram_tensor("wq", [D, HC * DK], BF16, kind="ExternalInput").ap()
    wk = nc.dram_tensor("wk", [D, HC * DK], BF16, kind="ExternalInput").ap()
    wv = nc.dram_tensor("wv", [D, HC * DV], BF16, kind="ExternalInput").ap()
    wo = nc.dram_tensor("wo", [HC * DV, D], BF16, kind="ExternalInput").ap()
    maskr = nc.dram_tensor("maskr", [128, NKC], F32, kind="ExternalInput").ap()
    out = nc.dram_tensor("out", [S, D], F32, kind="ExternalOutput").ap()

    with tile.TileContext(nc) as tc:
        with ExitStack() as ctx:
            const_pool = ctx.enter_context(tc.tile_pool(name="const", bufs=1))
            w_pool = ctx.enter_context(tc.tile_pool(name="weights", bufs=1))
            act_pool = ctx.enter_context(tc.tile_pool(name="acts", bufs=1))

            mask_sb = const_pool.tile([128, NKC], F32)
            ones_sb = const_pool.tile([128, HC], BF16)
            sel16 = const_pool.tile([1, DV], F16)

            # weights staged chunk-major: col block c holds rows c*128..+128
            wq_sb = w_pool.tile([128, NC_CHUNKS * 512], BF16, tag="wq")
            wv_sb = w_pool.tile([128, NC_CHUNKS * 512], BF16, tag="wv")
            wo_sb = w_pool.tile([128, NP * 1024], BF16, tag="wo")

            # persistent activations (pair-packed: rows 0-63 h0, 64-127 h1)
            qhT = [act_pool.tile([128, S], BF16, tag=f"qhT{p}", name=f"qhT{p}")
                   for p in range(NP)]
            khT = [act_pool.tile([128, S], BF16, tag=f"khT{p}", name=f"khT{p}")
                   for p in range(NP)]
            vhs = [act_pool.tile([128, VW], BF16, tag=f"vh{t}", name=f"vh{t}")
                   for t in range(NKC)]

            # attention pools created BEFORE the staging scopes so that the
            # staging scopes can close in LIFO order mid-kernel.
            qst_pool = ctx.enter_context(
                tc.tile_pool(name="qstage", bufs=12))
            sc_pool = ctx.enter_context(
                tc.tile_pool(name="scpsum", bufs=2, space="PSUM"))
            mx_pool = ctx.enter_context(
                tc.tile_pool(name="mxpsum", bufs=2, space="PSUM"))
            sh_pool = ctx.enter_context(
                tc.tile_pool(name="shpsum", bufs=2, space="PSUM"))
            exp_pool = ctx.enter_context(tc.tile_pool(name="exp", bufs=6))
            norm_pool = ctx.enter_context(tc.tile_pool(name="norm", bufs=9))
            tmp_pool = ctx.enter_context(tc.tile_pool(name="tmp", bufs=3))
            s64_pool = ctx.enter_context(tc.tile_pool(name="s64", bufs=2))
            srow_pool = ctx.enter_context(tc.tile_pool(name="srow", bufs=2))
            out_pool = ctx.enter_context(tc.tile_pool(name="outsb", bufs=2))

            # ---- issue all input DMAs up front (big tiles first) ----
            # vstage allocated first, kstage second: kstage scope closes
            # right after the prefix (LIFO), vstage after pair 0's v-proj.
            vst_ctx = ExitStack()
            vst_pool = vst_ctx.enter_context(
                tc.tile_pool(name="vstage", bufs=1))
            vstg = [vst_pool.tile([128, S], BF16, tag=f"vst{c}",
                                  name=f"vst{c}") for c in range(NC_CHUNKS)]
            kst_ctx = ExitStack()
            kst_pool = kst_ctx.enter_context(
                tc.tile_pool(name="kstage", bufs=1))
            wk_sb = kst_pool.tile([128, NC_CHUNKS * 512], BF16, tag="wk")
            kstg = [kst_pool.tile([128, S], BF16, tag=f"kst{c}",
                                  name=f"kst{c}") for c in range(NC_CHUNKS)]
            for c in range(NC_CHUNKS):
                nc.sync.dma_start(wk_sb[:, c * 512:(c + 1) * 512],
                                  wk[c * 128:(c + 1) * 128, :])
                nc.sync.dma_start(kstg[c][:], kT[c * 128:(c + 1) * 128, :])

            def stage_q(qb):
                stg = []
                for c in range(NC_CHUNKS):
                    t = qst_pool.tile([128, 512], BF16, tag="qst",
                                      name=f"qst{qb}_{c}")
                    nc.sync.dma_start(
                        t[:], qT[c * 128:(c + 1) * 128,
                                 qb * 512:(qb + 1) * 512])
                    stg.append(t)
                return stg

            for c in range(NC_CHUNKS):
                nc.sync.dma_start(wq_sb[:, c * 512:(c + 1) * 512],
                                  wq[c * 128:(c + 1) * 128, :])
            qstg0 = stage_q(0)
            nc.sync.dma_start(mask_sb[:], maskr[:])
            for c in range(NC_CHUNKS):
                nc.sync.dma_start(wv_sb[:, c * 512:(c + 1) * 512],
                                  wv[c * 128:(c + 1) * 128, :])
            for p in range(NP):
                nc.sync.dma_start(wo_sb[:, p * 1024:(p + 1) * 1024],
                                  wo[p * 128:(p + 1) * 128, :])
            for c in range(NC_CHUNKS):
                nc.sync.dma_start(vstg[c][:], vT[c * 128:(c + 1) * 128, :])
            nc.vector.memset(ones_sb[:], 1.0)
            nc.vector.memset(sel16[:], 1.0)

            # ---- prefix: k projection (c-outer, stationary reuse) ----
            # kps tiles come from the attention sc pool ([128,1024] holds
            # two query blocks side by side).
            for p in range(NP):
                kpsA = sc_pool.tile([128, 1024], F32, tag="sc",
                                    name=f"kpsA{p}")
                kpsB = sc_pool.tile([128, 1024], F32, tag="sc",
                                    name=f"kpsB{p}")
                halves = [kpsA[:, 0:512], kpsA[:, 512:1024],
                          kpsB[:, 0:512], kpsB[:, 512:1024]]
                for c in range(NC_CHUNKS):
                    wsl = wk_sb[:, c * 512 + p * 128:
                                c * 512 + (p + 1) * 128]
                    for qb in range(NQB):
                        nc.tensor.matmul(
                            halves[qb],
                            lhsT=wsl,
                            rhs=kstg[c][:, qb * 512:(qb + 1) * 512],
                            start=(c == 0), stop=(c == NC_CHUNKS - 1))
                nc.vector.tensor_copy(khT[p][:, 0:1024], kpsA[:])
                nc.vector.tensor_copy(khT[p][:, 1024:2048], kpsB[:])
            # q projection for qb0
            for p in range(NP):
                qps = sh_pool.tile([128, 512], F32, tag="sh",
                                   name=f"qps0_{p}")
                for c in range(NC_CHUNKS):
                    nc.tensor.matmul(
                        qps[:],
                        lhsT=wq_sb[:, c * 512 + p * 128:
                                   c * 512 + (p + 1) * 128],
                        rhs=qstg0[c][:],
                        start=(c == 0), stop=(c == NC_CHUNKS - 1))
                nc.vector.tensor_copy(qhT[p][:, 0:512], qps[:])
            kst_ctx.close()   # free kT staging (32KB/partition)

            def vproj_chunk(t):
                """Project v for token chunk t into vhs[t] (all 8 heads)."""
                vps = sh_pool.tile([128, 512], F32, tag="sh")
                for c in range(NC_CHUNKS):
                    nc.tensor.matmul(
                        vps[:],
                        lhsT=vstg[c][:, t * 128:(t + 1) * 128],
                        rhs=wv_sb[:, c * 512:(c + 1) * 512],
                        start=(c == 0), stop=(c == NC_CHUNKS - 1))
                dst_dv = vhs[t][:, 0:VW].rearrange(
                    "p (h x) -> p h x", x=65)[:, :, 0:DV]
                src_dv = vps[:].rearrange("p (h x) -> p h x", x=DV)
                nc.vector.tensor_scalar_mul(dst_dv, src_dv,
                                            mask_sb[:, t:t + 1])
                dst_m = vhs[t][:, 0:VW].rearrange(
                    "p (h x) -> p h x", x=65)[:, :, DV:DV + 1]
                src_m = ones_sb[:, 0:HC].rearrange("p (h x) -> p h x", x=1)
                nc.vector.tensor_scalar_mul(dst_m, src_m,
                                            mask_sb[:, t:t + 1])

            def qproj_group(qb, p, stg):
                """Project q for (qb, pair p) into qhT[p]."""
                qps = sh_pool.tile([128, 512], F32, tag="sh")
                for c in range(NC_CHUNKS):
                    nc.tensor.matmul(
                        qps[:],
                        lhsT=wq_sb[:, c * 512 + p * 128:
                                   c * 512 + (p + 1) * 128],
                        rhs=stg[c][:],
                        start=(c == 0), stop=(c == NC_CHUNKS - 1))
                nc.vector.tensor_copy(qhT[p][:, qb * 512:(qb + 1) * 512],
                                      qps[:])

            def wo_group(qb, tt, dh, normT):
                """One Wo output tile [128 q, 512 d] accumulated over pairs."""
                wps = sh_pool.tile([128, 512], F32, tag="sh")
                for p in range(NP):
                    nc.tensor.matmul(
                        wps[:],
                        lhsT=normT[p][:, tt * 128:(tt + 1) * 128],
                        rhs=wo_sb[:, p * 1024 + dh * 512:
                                  p * 1024 + (dh + 1) * 512],
                        start=(p == 0), stop=(p == NP - 1))
                osb = out_pool.tile([128, 512], F32, tag="osb")
                if qb == NQB - 1:
                    nc.scalar.copy(osb[:], wps[:])
                else:
                    nc.vector.tensor_copy(osb[:], wps[:])
                nc.sync.dma_start(
                    out[qb * 512 + tt * 128:qb * 512 + (tt + 1) * 128,
                        dh * 512:(dh + 1) * 512], osb[:])

            # interleave slot queue: list of thunks
            pending = []
            post_norm = []

            def run_slot():
                if pending:
                    pending.pop(0)()

            for qb in range(NQB):
                normT = []
                for p in range(NP):
                    first = (qb == 0 and p == 0)
                    lag = 4 if first else 3
                    if p == NP - 1 and qb + 1 < NQB:
                        # queue q-projection for qb+1 into this pair's slots
                        stg = stage_q(qb + 1)
                        for pp in range(NP):
                            pending.append(
                                lambda qb=qb, pp=pp, stg=stg: qproj_group(
                                    qb + 1, pp, stg))
                    h0, h1 = 2 * p, 2 * p + 1
                    qful = qhT[p][:, qb * 512:(qb + 1) * 512]
                    mixP = mx_pool.tile([128, 512], F32, tag="mx")
                    mixR = mx_pool.tile([128, 512], F32, tag="mx")
                    exps = []

                    def mix_step(lk):
                        for h, mx in ((h0, mixP), (h1, mixR)):
                            nc.tensor.matmul(
                                mx[0:65, :],
                                lhsT=vhs[lk][:, h * 65:h * 65 + 65],
                                rhs=exps[lk][:, (h % 2) * 512:
                                             (h % 2) * 512 + 512],
                                start=(lk == 0), stop=(lk == NKC - 1))

                    # scores + exp per key chunk (two K=64 row tiles)
                    for kc in range(NKC):
                        ksl = slice(kc * 128, (kc + 1) * 128)
                        sc = sc_pool.tile([128, 1024], F32, tag="sc")
                        nc.tensor.matmul(
                            sc[:, 0:512],
                            lhsT=khT[p][0:64, ksl], rhs=qful[0:64, :],
                            start=True, stop=True)
                        nc.tensor.matmul(
                            sc[:, 512:1024],
                            lhsT=khT[p][64:128, ksl], rhs=qful[64:128, :],
                            start=True, stop=True)
                        ex = exp_pool.tile([128, 1024], BF16, tag="exp")
                        nc.scalar.activation(ex[:], sc[:], EXP)
                        exps.append(ex)
                        if first:
                            # v projection rides pair 0's score loop
                            vproj_chunk(kc)
                        if kc == 5 and post_norm:
                            post_norm.pop(0)()
                        if kc >= lag:
                            mix_step(kc - lag)
                        if (not first and kc % 4 == 3
                                and not (p == 0 and kc == 3)):
                            run_slot()
                    for lk in range(NKC - lag, NKC):
                        mix_step(lk)
                    if first:
                        vst_ctx.close()   # free vT staging
                    # ---- normalize (pre): sums rows -> fp16 -> partition 0
                    s64 = s64_pool.tile([128, 1024], F16, tag="s64")
                    nc.vector.tensor_copy(s64[64:65, 0:512], mixP[64:65, :])
                    nc.vector.tensor_copy(s64[64:65, 512:1024],
                                          mixR[64:65, :])
                    srow = srow_pool.tile([1, 1024], F16, tag="srow")
                    nc.sync.dma_start(srow[0:1, :], s64[64:65, :])

                    def norm_post(mixP=mixP, mixR=mixR, srow=srow,
                                  normT=normT):
                        # broadcast (K=1 fp16 matmuls), recip, multiply;
                        # deferred into the next pair so the PE never waits
                        # on the sums DMA.
                        bc0 = sh_pool.tile([128, 512], F32, tag="sh")
                        bc1 = sh_pool.tile([128, 512], F32, tag="sh")
                        nc.tensor.matmul(bc0[0:DV, :], lhsT=sel16[:],
                                         rhs=srow[0:1, 0:512],
                                         start=True, stop=True)
                        nc.tensor.matmul(bc1[0:DV, :], lhsT=sel16[:],
                                         rhs=srow[0:1, 512:1024],
                                         start=True, stop=True)
                        rec0 = tmp_pool.tile([64, 512], F32, tag="rec")
                        rec1 = tmp_pool.tile([64, 512], F32, tag="rec")
                        nc.vector.reciprocal_approx_fast(rec0[:],
                                                         bc0[0:64, :])
                        nc.vector.reciprocal_approx_fast(rec1[:],
                                                         bc1[0:64, :])
                        nt = norm_pool.tile([128, 512], BF16, tag="norm")
                        normT.append(nt)
                        nc.vector.tensor_mul(nt[0:64, :], mixP[0:64, :],
                                             rec0[:])
                        sh1 = tmp_pool.tile([64, 512], BF16, tag="sh1")
                        nc.vector.tensor_mul(sh1[:], mixR[0:64, :], rec1[:])
                        nc.sync.dma_start(nt[64:128, :], sh1[:])

                    if qb == NQB - 1 and p == NP - 1:
                        norm_post()
                    else:
                        post_norm.append(norm_post)

                # queue Wo for this qb into the next qb's interleave slots
                # (normT is filled lazily by deferred norm_post thunks; pass
                # the live list, complete by the time any wo_group runs)
                nt_list = normT
                for tt in range(4):
                    for dh in range(2):
                        pending.append(
                            lambda qb=qb, tt=tt, dh=dh, nt=nt_list: wo_group(
                                qb, tt, dh, nt))
                # last qb: drain all pending now
                if qb == NQB - 1:
                    while pending:
                        run_slot()

    nc.compile()
    return nc


def _get_nc():
    if "nc" not in _COMPILED:
        _COMPILED["nc"] = _build_nc()
    return _COMPILED["nc"]


def _shard_inputs(q, k, v, mask, Wq, Wk, Wv, Wo):
    """Build the per-core input maps (host-side layout prep)."""
    import ml_dtypes

    bf16 = ml_dtypes.bfloat16
    in_maps = []
    maskf = np.asarray(mask).astype(np.float32)
    q = np.asarray(q, np.float32)
    k = np.asarray(k, np.float32)
    v = np.asarray(v, np.float32)
    Wq = np.asarray(Wq, np.float32)
    Wk = np.asarray(Wk, np.float32)
    Wv = np.asarray(Wv, np.float32)
    Wo = np.asarray(Wo, np.float32)
    scale = np.float32(1.0 / np.sqrt(DK))
    for c in range(NCORES):
        b, hg = c // 2, c % 2
        hs = hg * HC
        m = {
            "qT": np.ascontiguousarray(q[b].T).astype(bf16),
            "kT": np.ascontiguousarray(k[b].T).astype(bf16),
            "vT": np.ascontiguousarray(v[b].T).astype(bf16),
            # head-major col blocks; fold 1/sqrt(dk) into Wq
            "wq": np.ascontiguousarray(
                Wq[hs:hs + HC].transpose(1, 0, 2).reshape(D, HC * DK) * scale
            ).astype(bf16),
            "wk": np.ascontiguousarray(
                Wk[hs:hs + HC].transpose(1, 0, 2).reshape(D, HC * DK)
            ).astype(bf16),
            "wv": np.ascontiguousarray(
                Wv[hs:hs + HC].transpose(1, 0, 2).reshape(D, HC * DV)
            ).astype(bf16),
            "wo": np.ascontiguousarray(Wo[hs * DV:(hs + HC) * DV]).astype(bf16),
            "maskr": np.ascontiguousarray(
                maskf[b].reshape(NKC, 128).T).astype(np.float32),
        }
        in_maps.append(m)
    return in_maps


def kernel(q, k, v, mask, Wq, Wk, Wv, Wo, _trace=False):
    from concourse.bass_utils import run_bass_kernel_spmd

    nc = _get_nc()
    in_maps = _shard_inputs(q, k, v, mask, Wq, Wk, Wv, Wo)
    res = run_bass_kernel_spmd(nc, in_maps, list(range(NCORES)),
                               trace=_trace)
    out = np.zeros((B, S, D), np.float32)
    for c in range(NCORES):
        out[c // 2] += res.results[c]["out"]
    if _trace:
        _COMPILED["last_result"] = res
    return out
